# revision 49
# baseline (speedup 1.0000x reference)
"""Trainium2 Bass kernel for nn_AnimatingSoftmaxSplating (depth2mesh).

The reference returns (v, tris, mask) where, with the default arguments
MAX_COS=2.0 and MAX_LEN=-1:
  * v[b, n, c] = xyz[c, n] * D[b, n]  (xyz = normalized pixel grid, row 2 = 1)
  * tris[:, 0] / tris[:, 1] are input-independent constant index tables
  * tris[:, 2, q] = quad[(J+1+k) % 4, q] where J = index of the first invalid
    quad corner (D <= EPS), 0 if none
  * mask[:, 0] and mask[:, 1] are always True: every entry of tri_cos is a
    cosine bounded by 1 in magnitude (Cauchy-Schwarz, denominators clamped to
    EPS), so tri_cos < 2.0 holds identically
  * mask[:, 2] = (number of valid quad corners == 3)

With prefix products of the corner-validity bits A=qv0, B=A*qv1, C=B*qv2,
E=C*qv3 and s1=E-B, s2=C-A, the data-dependent triangle row reduces to
  tris[:, 2, q, k] = i*W + bias_k + inner_k(j),   with
  inner0 = W*s1 - s2 + j,  inner1 = W*s2 + s1 + j,  inner2 = 2j - inner0
(verified exhaustively over all 16 validity patterns).

Sharding: data-parallel over the batch dimension; each of the 8 NeuronCores
processes one batch element.

Implementation notes (raw Bass, hand-scheduled):
  * This toolchain accepts only ONE inline sync-wait per instruction, so all
    multi-producer joins are standalone wait_ge instructions on the consuming
    engine's sequencer.
  * The depth map is host-padded to 513 rows so the plain and row-shifted
    halves are each a single straight DMA from DRAM; the algebra is split
    into two row-halves so compute starts as soon as the first half lands.
  * Mask algebra runs on the Vector engine in fp16 (all values are small
    integers, exact in fp16), 2 row-chunks side by side in the free axis.
    tensor_scalar (2x/4x perf modes) + tensor_tensor (2x) pairs are used
    instead of scalar_tensor_tensor, which only has a 1x uop.
  * The triangle planes are per-partition bias adds, split: ScalarE does
    chunks 0-2 (pipelined per-k against the inner-chain semaphore ticks),
    VectorE does chunk 3, so the post-chain tail is short.
  * GPSIMD generates the j-index constants (iota + 2j), computes v channel 0
    (xn*D), the mask-row sums, and the first mask half; ScalarE writes v
    channels 1 (yn*D via activation scale) and 2 (copy); v is emitted bf16
    (max elementwise error 2^-9, ~100x under the 2e-2 gate).
  * Outputs stream by readiness: v chunks early, the first mask half as soon
    as GPSIMD finishes it, triangle chunks as each chunk's planes finish,
    the second mask half last (512 B-padded rows throughout).
"""

import sys

import numpy as np

if "/opt/trn_rl_repo" not in sys.path:  # concourse (Bass) lives here
    sys.path.insert(0, "/opt/trn_rl_repo")

W = 512
N = W * W
WM = W - 1  # 511
Q = WM * WM
EPS = 1e-6
NCORES = 8
FW = 4 * W       # 2048: free width of full-D tiles (4 row-chunks side by side)
M = FW - 1       # 2047: width of shifted-product tiles
QW = 4 * WM      # 2044: packed quad-column width (chunk-major c*511 + j)

_compiled = {}


def _host_consts():
    """Input-independent constant arrays (host side)."""
    j = np.arange(W, dtype=np.float32)
    # xn[j] = (j/(W-1))*2 - 1, same f32 op sequence as the reference; the row
    # coordinate yn uses the identical formula.
    xn = (j / np.float32(W - 1)) * np.float32(2.0) - np.float32(1.0)
    rows = (np.arange(4)[None, :] * 128 + np.arange(128)[:, None]).astype(np.float32)
    yn4 = (rows / np.float32(W - 1)) * np.float32(2.0) - np.float32(1.0)

    # biases[p, k*4 + c] = (c*128 + p)*W + off_k, off = [W, W+1, 1]
    r = np.arange(4)[None, :] * 128 + np.arange(128)[:, None]
    biases = np.empty((128, 12), dtype=np.float32)
    for k, off in enumerate((W, W + 1, 1)):
        biases[:, k * 4 : (k + 1) * 4] = (r * W + off).astype(np.float32)

    # consts_f32 layout: [xn (512) | yn4 (4) | biases (12)] -> [128, 528]
    consts_f32 = np.empty((128, W + 4 + 12), dtype=np.float32)
    consts_f32[:, 0:W] = xn[None, :]
    consts_f32[:, W : W + 4] = yn4
    consts_f32[:, W + 4 : W + 16] = biases
    consts_f32 = np.ascontiguousarray(consts_f32)

    # constant triangle tables (rows 0 and 1 of tris)
    y, x = np.meshgrid(np.arange(W, dtype=np.int32), np.arange(W, dtype=np.int32),
                       indexing="ij")
    lin = x + y * W
    quad = np.stack([
        lin[:-1, :-1].reshape(-1),
        lin[1:, :-1].reshape(-1),
        lin[1:, 1:].reshape(-1),
        lin[:-1, 1:].reshape(-1),
    ], axis=0).astype(np.int32)                      # [4, Q]
    tri_a = np.ascontiguousarray(quad[[0, 1, 2]].T)  # [Q, 3]
    tri_b = np.ascontiguousarray(quad[[2, 3, 0]].T)  # [Q, 3]
    return consts_f32, tri_a, tri_b


def _build_bass():
    from contextlib import ExitStack

    import concourse.bass as bass
    from concourse import mybir

    AO = mybir.AluOpType
    AF = mybir.ActivationFunctionType
    F16 = mybir.dt.float16
    F32 = mybir.dt.float32

    nc = bass.Bass()
    # d is host-padded with one junk row (value 1.0) to 513 rows.
    d = nc.declare_dram_parameter("d", [W + 1, W], F32, isOutput=False)
    cf32 = nc.declare_dram_parameter("cf32", [128, W + 16], F32, isOutput=False)
    v_out = nc.declare_dram_parameter("v_out", [W, 3 * W], mybir.dt.bfloat16,
                                      isOutput=True)
    t_out = nc.declare_dram_parameter("t_out", [WM, 3 * WM], mybir.dt.int32,
                                      isOutput=True)
    # mask rows padded to 512 B (and one padding row) so every DMA segment is
    # >= 512 B; the host strips the padding
    m_out = nc.declare_dram_parameter("m_out", [W, W], mybir.dt.uint8,
                                      isOutput=True)

    with ExitStack() as ctx:
        def sb(name, shape, dt_):
            return ctx.enter_context(nc.sbuf_tensor(name, shape, dt_))

        dall = sb("dall", [128, FW], F32)
        dallb = sb("dallb", [128, FW], F32)
        csb32 = sb("csb32", [128, W + 16], F32)
        csb16 = sb("csb16", [128, QW], mybir.dt.int16)  # j per quad col (iota)
        dva = sb("dva", [128, FW], F16)
        dvb = sb("dvb", [128, FW], F16)
        rs = sb("rs", [128, FW], F16)
        rp = sb("rp", [128, FW], F16)
        ep = sb("ep", [128, FW], F16)   # rp - 1
        cp = sb("cp", [128, M], F16)
        s1p = sb("s1p", [128, QW], F16)
        s2p = sb("s2p", [128, QW], F16)
        qs = sb("qs", [128, M], F16)
        j24 = sb("j24", [128, QW], F16)
        s2jm = sb("s2jm", [128, QW], F16)
        s1j = sb("s1j", [128, QW], F16)
        ws1 = sb("ws1", [128, QW], F16)
        ws2 = sb("ws2", [128, QW], F16)
        i0 = sb("i0", [128, QW], F16)
        i1 = sb("i1", [128, QW], F16)
        i2 = sb("i2", [128, QW], F16)
        om = sb("om", [128, FW], mybir.dt.uint8)   # chunk-major c*512 + j
        ovc = [sb(f"ovc{c}", [128, 3 * W], mybir.dt.bfloat16) for c in range(4)]
        otc = [sb(f"otc{c}", [128, 3 * WM], mybir.dt.int32) for c in range(4)]

        sem_a1 = ctx.enter_context(nc.semaphore("sem_a1"))    # dall half A
        sem_b1 = ctx.enter_context(nc.semaphore("sem_b1"))    # dallb half A
        sem_a2 = ctx.enter_context(nc.semaphore("sem_a2"))    # dall half B
        sem_b2 = ctx.enter_context(nc.semaphore("sem_b2"))    # dallb half B
        sem_c32 = ctx.enter_context(nc.semaphore("sem_c32"))  # cf32 load
        sem_it = ctx.enter_context(nc.semaphore("sem_it"))    # Pool iota done
        sem_pj = ctx.enter_context(nc.semaphore("sem_pj"))    # Pool j24 done
        sem_cmp = ctx.enter_context(nc.semaphore("sem_cmp"))  # DVE cmps/half
        sem_iA = ctx.enter_context(nc.semaphore("sem_iA"))    # A inners (3 ticks)
        sem_iB = ctx.enter_context(nc.semaphore("sem_iB"))    # B inners (3 ticks)
        sem_q = ctx.enter_context(nc.semaphore("sem_q"))      # Pool qs halves
        sem_ms = ctx.enter_context(nc.semaphore("sem_ms"))    # om memset done
        sem_mA = ctx.enter_context(nc.semaphore("sem_mA"))    # om half A (Pool)
        sem_m = ctx.enter_context(nc.semaphore("sem_m"))      # om done (DVE)
        sem_v = ctx.enter_context(nc.semaphore("sem_v"))      # Pool v0 per chunk
        sem_av = ctx.enter_context(nc.semaphore("sem_av"))    # ACT v1/v2 ops
        sem_t = ctx.enter_context(nc.semaphore("sem_t"))      # ACT planes/chunk
        sem_t3 = ctx.enter_context(nc.semaphore("sem_t3"))    # DVE planes c3
        sem_out = ctx.enter_context(nc.semaphore("sem_out"))  # out-DMA tally

        block = ctx.enter_context(nc.Block())

        xn_sb = csb32[:, 0:W]
        yn_sb = csb32[:, W : W + 4]
        bias_sb = csb32[:, W + 4 : W + 16]
        jr4 = csb16[:, 0:QW]

        H = 2 * W    # full-tile half width (chunks 0,1)
        HQ = 2 * WM  # packed half width

        def halfview(tile, half, shift=0):
            """[[512,2],[1,511]] view: cols half*1024 + c*512 + j + shift."""
            ap = tile[:, :]
            return bass.AP(tensor=ap.tensor,
                           offset=ap.offset + half * H + shift,
                           ap=[ap.ap[0], [W, 2], [1, WM]])

        @block.vector
        def _(vector):
            for h in range(2):  # half A = chunks 0,1; half B = chunks 2,3
                b = h * H
                bq = h * HQ
                vector.wait_ge(sem_a1 if h == 0 else sem_a2, 16)
                vector.tensor_scalar(out=dva[:, b : b + H],
                                     in0=dall[:, b : b + H],
                                     scalar1=float(EPS), scalar2=None,
                                     op0=AO.is_gt)
                vector.wait_ge(sem_b1 if h == 0 else sem_b2, 16)
                vector.tensor_scalar(out=dvb[:, b : b + H],
                                     in0=dallb[:, b : b + H],
                                     scalar1=float(EPS), scalar2=None,
                                     op0=AO.is_gt).then_inc(sem_cmp, 1)
                # row-pair product rp = qv0*qv1 per column; rp-1 feeds the
                # fused s1 = E - B = B*(rp_{j+1} - 1). Chunk-boundary columns
                # are garbage, never consumed. (rs lives on Pool.)
                vector.tensor_tensor(out=rp[:, b : b + H],
                                     in0=dva[:, b : b + H],
                                     in1=dvb[:, b : b + H], op=AO.mult)
                vector.tensor_scalar(out=ep[:, b : b + H],
                                     in0=rp[:, b : b + H], scalar1=-1.0,
                                     scalar2=None, op0=AO.add)  # rp - 1
                vector.tensor_tensor(out=s1p[:, bq : bq + HQ],
                                     in0=halfview(rp, h),
                                     in1=halfview(ep, h, shift=1),
                                     op=AO.mult)
                vector.tensor_tensor(out=cp[:, b : b + H - 1],
                                     in0=rp[:, b : b + H - 1],
                                     in1=dvb[:, b + 1 : b + H], op=AO.mult)
                vector.tensor_tensor(out=s2p[:, bq : bq + HQ],
                                     in0=halfview(cp, h),
                                     in1=halfview(dva, h), op=AO.subtract)
                if h == 0:
                    vector.wait_ge(sem_it, 1)
                # inner chain: tensor_scalar (2x/4x modes) + tensor_tensor
                # (2x) pairs beat the 1x-only scalar_tensor_tensor
                vector.tensor_tensor(               # j - s2
                    out=s2jm[:, bq : bq + HQ], in0=jr4[:, bq : bq + HQ],
                    in1=s2p[:, bq : bq + HQ], op=AO.subtract)
                vector.tensor_scalar(               # W*s1
                    out=ws1[:, bq : bq + HQ], in0=s1p[:, bq : bq + HQ],
                    scalar1=float(W), scalar2=None, op0=AO.mult)
                sem_i = sem_iA if h == 0 else sem_iB
                vector.tensor_tensor(               # i0 = W*s1 + (j - s2)
                    out=i0[:, bq : bq + HQ], in0=ws1[:, bq : bq + HQ],
                    in1=s2jm[:, bq : bq + HQ], op=AO.add).then_inc(sem_i, 1)
                vector.tensor_tensor(               # s1 + j
                    out=s1j[:, bq : bq + HQ], in0=s1p[:, bq : bq + HQ],
                    in1=jr4[:, bq : bq + HQ], op=AO.add)
                vector.tensor_scalar(               # W*s2
                    out=ws2[:, bq : bq + HQ], in0=s2p[:, bq : bq + HQ],
                    scalar1=float(W), scalar2=None, op0=AO.mult)
                vector.tensor_tensor(               # i1 = W*s2 + (s1 + j)
                    out=i1[:, bq : bq + HQ], in0=ws2[:, bq : bq + HQ],
                    in1=s1j[:, bq : bq + HQ], op=AO.add).then_inc(sem_i, 1)
                if h == 0:
                    vector.wait_ge(sem_pj, 1)
                vector.tensor_tensor(               # i2 = 2j - i0
                    out=i2[:, bq : bq + HQ], in0=j24[:, bq : bq + HQ],
                    in1=i0[:, bq : bq + HQ],
                    op=AO.subtract).then_inc(sem_i, 1)
            # triangle planes for chunk 3 (127 valid rows) on DVE
            vector.wait_ge(sem_c32, 16)
            for k, it in enumerate((i0, i1, i2)):
                ins = vector.tensor_scalar(
                    out=otc[3][:127, k : 3 * WM : 3],
                    in0=it[:127, 3 * WM : 4 * WM],
                    scalar1=bias_sb[:127, k * 4 + 3 : k * 4 + 4],
                    scalar2=None, op0=AO.add)
                if k == 2:
                    ins.then_inc(sem_t3, 1)
            # mask row 2 (quad_valid == 3), half B only (Pool owns half A)
            vector.wait_ge(sem_ms, 1)
            vector.wait_ge(sem_q, 1)
            vector.tensor_scalar(
                out=bass.AP(tensor=om[:, :].tensor,
                            offset=om[:, :].offset + H,
                            ap=[om[:, :].ap[0], [W, 2], [1, WM]]),
                in0=halfview(qs, 1), scalar1=3.0, scalar2=None,
                op0=AO.is_equal).then_inc(sem_m, 1)

        @block.scalar
        def _(scalar):
            scalar.wait_ge(sem_a1, 16)
            scalar.wait_ge(sem_c32, 16)
            for c in range(4):
                if c == 2:
                    scalar.wait_ge(sem_a2, 16)
                b = c * W
                scalar.activation(                   # v channel 1 = yn * D
                    out=ovc[c][:, 1 : 3 * W : 3], in_=dall[:, b : b + W],
                    func=AF.Copy, bias=0.0,
                    scale=yn_sb[:, c : c + 1]).then_inc(sem_av, 1)
                scalar.activation(                   # v channel 2 = D
                    out=ovc[c][:, 2 : 3 * W : 3], in_=dall[:, b : b + W],
                    func=AF.Copy, bias=0.0, scale=1.0).then_inc(sem_av, 1)
            # triangle planes (AP bias via activation; Relu is identity on
            # these values >= 0): chunks 0,1 after the A half; chunk 2 tracks
            # the B-half inner ticks so its planes pipeline with the chain
            for c in (0, 1):
                bq = c * WM
                for k, it in enumerate((i0, i1, i2)):
                    if c == 0:
                        scalar.wait_ge(sem_iA, k + 1)
                    ins = scalar.activation(
                        out=otc[c][:, k : 3 * WM : 3],
                        in_=it[:, bq : bq + WM], func=AF.Relu,
                        bias=bias_sb[:, k * 4 + c : k * 4 + c + 1], scale=1.0)
                    if k == 2:
                        ins.then_inc(sem_t, 1)
            for k, it in enumerate((i0, i1, i2)):
                scalar.wait_ge(sem_iB, k + 1)
                ins = scalar.activation(
                    out=otc[2][:, k : 3 * WM : 3],
                    in_=it[:, 2 * WM : 3 * WM], func=AF.Relu,
                    bias=bias_sb[:, k * 4 + 2 : k * 4 + 3], scale=1.0)
                if k == 2:
                    ins.then_inc(sem_t, 1)

        @block.gpsimd
        def _(gpsimd):
            gpsimd.iota(csb16[:], pattern=[[0, 4], [1, WM]], base=0,
                        channel_multiplier=0).then_inc(sem_it, 1)
            gpsimd.tensor_scalar(out=j24[:], in0=jr4, scalar1=2.0,
                                 scalar2=None, op0=AO.mult).then_inc(sem_pj, 1)
            gpsimd.memset(om[:], 0).then_inc(sem_ms, 1)
            # v channel 0 = xn * D (frees DVE; Pool is otherwise idle early)
            gpsimd.wait_ge(sem_a1, 16)
            gpsimd.wait_ge(sem_c32, 16)
            for c in range(4):
                if c == 2:
                    gpsimd.wait_ge(sem_a2, 16)
                b = c * W
                gpsimd.tensor_tensor(
                    out=ovc[c][:, 0 : 3 * W : 3], in0=dall[:, b : b + W],
                    in1=xn_sb, op=AO.mult).then_inc(sem_v, 1)
            # mask inputs: rs = qv0 + qv1, qs = rs_j + rs_{j+1}; half A's
            # om is computed here too (it is ready ~5 us before half B)
            for h in range(2):
                b = h * H
                gpsimd.wait_ge(sem_cmp, h + 1)
                gpsimd.tensor_tensor(out=rs[:, b : b + H],
                                     in0=dva[:, b : b + H],
                                     in1=dvb[:, b : b + H], op=AO.add)
                ins = gpsimd.tensor_tensor(out=qs[:, b : b + H - 1],
                                     in0=rs[:, b : b + H - 1],
                                     in1=rs[:, b + 1 : b + H], op=AO.add)
                if h == 1:
                    ins.then_inc(sem_q, 1)
                if h == 0:
                    gpsimd.wait_ge(sem_ms, 1)
                    gpsimd.tensor_scalar(
                        out=bass.AP(tensor=om[:, :].tensor,
                                    offset=om[:, :].offset,
                                    ap=[om[:, :].ap[0], [W, 2], [1, WM]]),
                        in0=halfview(qs, 0), scalar1=3.0, scalar2=None,
                        op0=AO.is_equal).then_inc(sem_mA, 1)

        @block.sync
        def _(sync):
            dt_ = d[:, :].tensor
            half = [[W, 128], [W * 128, 2], [1, W]]
            sync.dma_start(out=dall[:, 0:H], in_=bass.AP(
                tensor=dt_, offset=0, ap=half)).then_inc(sem_a1, 16)
            sync.dma_start(out=dallb[:, 0:H], in_=bass.AP(
                tensor=dt_, offset=W, ap=half)).then_inc(sem_b1, 16)
            sync.dma_start(out=csb32[:], in_=cf32[:, :]).then_inc(sem_c32, 16)
            sync.dma_start(out=dall[:, H:FW], in_=bass.AP(
                tensor=dt_, offset=256 * W, ap=half)).then_inc(sem_a2, 16)
            sync.dma_start(out=dallb[:, H:FW], in_=bass.AP(
                tensor=dt_, offset=257 * W, ap=half)).then_inc(sem_b2, 16)
            # stores, in approximate readiness order
            for c in range(4):
                sync.wait_ge(sem_v, c + 1)
                sync.wait_ge(sem_av, 2 * (c + 1))
                sync.dma_start(out=v_out[c * 128 : (c + 1) * 128, :],
                               in_=ovc[c][:]).then_inc(sem_out, 16)
            sync.wait_ge(sem_t, 1)
            sync.dma_start(out=t_out[0:128, :],
                           in_=otc[0][:]).then_inc(sem_out, 16)
            sync.wait_ge(sem_t, 2)
            sync.dma_start(out=t_out[128:256, :],
                           in_=otc[1][:]).then_inc(sem_out, 16)
            mo = m_out[:, :]
            sync.wait_ge(sem_mA, 1)
            sync.dma_start(out=bass.AP(     # mask rows 0..255, 512 B segments
                tensor=mo.tensor, offset=0,
                ap=[[W, 128], [W * 128, 2], [1, W]]),
                in_=om[:, 0:H]).then_inc(sem_out, 16)
            sync.wait_ge(sem_t3, 1)
            sync.dma_start(out=t_out[384 : 384 + 127, :],
                           in_=otc[3][0:127, :]).then_inc(sem_out, 16)
            sync.wait_ge(sem_t, 3)
            sync.dma_start(out=t_out[256:384, :],
                           in_=otc[2][:]).then_inc(sem_out, 16)
            sync.wait_ge(sem_m, 1)
            sync.dma_start(out=bass.AP(     # mask rows 256..511 (511 junk)
                tensor=mo.tensor, offset=256 * W,
                ap=[[W, 128], [W * 128, 2], [1, W]]),
                in_=om[:, H:FW]).then_inc(sem_out, 16)
            sync.wait_ge(sem_out, 10 * 16)
    return nc


def _get_compiled():
    if "nc" not in _compiled:
        _compiled["nc"] = _build_bass()
    return _compiled["nc"]


def _consts_cached():
    if "consts" not in _compiled:
        _compiled["consts"] = _host_consts()
    return _compiled["consts"]


def run_on_device(D, trace=False, **trace_kwargs):
    """Run the SPMD kernel on 8 NeuronCores. D: [8, 1, W, W] float32.
    Returns (per_core_results, BassKernelResults)."""
    from concourse.bass_utils import run_bass_kernel_spmd

    consts_f32, _, _ = _consts_cached()
    nc = _get_compiled()
    in_maps = []
    for i in range(NCORES):
        dpad = np.empty((W + 1, W), dtype=np.float32)
        dpad[:W] = D[i, 0]
        dpad[W] = 1.0  # junk row; > EPS so downstream bits stay defined
        in_maps.append({
            "d": dpad,
            "cf32": consts_f32,
        })
    res = run_bass_kernel_spmd(nc, in_maps, core_ids=list(range(NCORES)),
                               trace=trace, **trace_kwargs)
    return res.results, res


def kernel(D):
    D = np.asarray(D, dtype=np.float32)
    assert D.shape == (NCORES, 1, W, W), D.shape
    results, _ = run_on_device(D)

    _, tri_a, tri_b = _consts_cached()

    v = np.empty((NCORES, N, 3), dtype=np.float32)
    tris = np.empty((NCORES, 3, Q, 3), dtype=np.int32)
    mask = np.empty((NCORES, 3, Q), dtype=np.bool_)
    for i in range(NCORES):
        r = results[i]
        v[i] = np.asarray(r["v_out"]).astype(np.float32).reshape(N, 3)
        tris[i, 0] = tri_a
        tris[i, 1] = tri_b
        tris[i, 2] = r["t_out"].reshape(Q, 3)
        mask[i, 0] = True
        mask[i, 1] = True
        # m_out is [512, 512] with one padding row and a padding column
        mask[i, 2] = r["m_out"][:WM, :WM].reshape(Q).astype(np.bool_)
    return v, tris, mask
